# revision 19
# baseline (speedup 1.0000x reference)
"""Trainium2 Bass kernel for ragged masked attention-score softmax.

Problem (B=32, T=8192, H=128):
    energy[b,t] = relu(W1 @ hidden[b] + W2 @ enc[t,b] + b_attn)   (W_attn = [W1 | W2])
    scores[b,t] = v . energy[b,t]
    out[b,0,:]  = ragged-masked softmax over t < len_seq[b], zeros after.

Strategy (8 NeuronCores, data-parallel over B):
  - Rows are sorted by len desc; slot j on every core takes one row from rank
    group [8j, 8j+8).  The per-slot static position count NP_j = max len in the
    group (rounded to 128), so all cores run one shared graph while skipping
    ~half the positions (ragged lengths are known at trace time).
  - Host passes each core's rows TRANSPOSED ([H, NP_j], H on partitions) so the
    device streams contiguous tiles straight into the TensorEngine with no
    on-device transpose of the big tensor.
  - Per 512-column tile: energy = W2T.T @ encT (PE) -> relu+hproj bias
    (ScalarE, PSUM->SBUF) -> v-dot via PE (energy tile as stationary, v as
    moving operand) accumulating scores[t,1] columns into a PSUM scores tile.
  - Epilogue: masked softmax on the [128, 4, T/128] scores tile (exact max via
    gpsimd partition_all_reduce), PE transpose per slot, DMA out.
"""

from contextlib import ExitStack

import numpy as np

import concourse.bass as bass
import concourse.tile as tile
from concourse import bacc, mybir
from concourse.bass_utils import run_bass_kernel_spmd

B, T, H = 32, 8192, 128
NCORES = 8
SLOTS = B // NCORES  # 4 rows per core
NEG = -1.0e30

# knobs
COMPUTE_DTYPE = "bfloat16"  # "float32" | "bfloat16" for enc/W2/v/energy matmul path
CHUNK = 4096  # positions per DMA (4096 bf16 = 1 MiB)
GRP = 1024  # relu granularity (2 PSUM banks)
MMN = 512  # matmul moving free dim (one PSUM bank in f32)


def _np_dt(name):
    if name == "bfloat16":
        import ml_dtypes

        return np.dtype(ml_dtypes.bfloat16)
    return np.dtype(np.float32)


def _my_dt(name):
    return mybir.dt.bfloat16 if name == "bfloat16" else mybir.dt.float32


def _plan(ls, t_max):
    """Assign rows to (core, slot). Returns rows[core][slot] = b, NP[slot]."""
    order = np.argsort(-np.asarray(ls), kind="stable")
    rows = [[int(order[8 * j + i]) for j in range(SLOTS)] for i in range(NCORES)]
    NP = []
    for j in range(SLOTS):
        mx = int(max(ls[int(order[8 * j + i])] for i in range(NCORES)))
        NP.append(min(((mx + 127) // 128) * 128, t_max))
    return rows, NP


def _build(nc, NP, nt_out, dt_name):
    """Emit the Tile graph. NP: per-slot position counts (mult of 128).
    nt_out: number of 128-wide t-tiles in the output (T/128)."""
    dt_c = _my_dt(dt_name)
    f32 = mybir.dt.float32
    AF = mybir.ActivationFunctionType

    encs = [
        nc.dram_tensor(f"enc{j}", [H, NP[j]], dt_c, kind="ExternalInput").ap()
        for j in range(SLOTS)
    ]
    hid = nc.dram_tensor("hid", [H, SLOTS], f32, kind="ExternalInput").ap()
    w1t = nc.dram_tensor("w1t", [H, H], f32, kind="ExternalInput").ap()
    w2t = nc.dram_tensor("w2t", [H, H], dt_c, kind="ExternalInput").ap()
    bvec = nc.dram_tensor("bvec", [H, 1], f32, kind="ExternalInput").ap()
    vvec = nc.dram_tensor("vvec", [H, 1], dt_c, kind="ExternalInput").ap()
    maskt = nc.dram_tensor("maskt", [128, SLOTS, nt_out], f32, kind="ExternalInput").ap()
    ident = nc.dram_tensor("ident", [128, 128], f32, kind="ExternalInput").ap()
    out = nc.dram_tensor("out", [SLOTS, nt_out, 128], f32, kind="ExternalOutput").ap()

    with ExitStack() as ctx:
        tc = ctx.enter_context(tile.TileContext(nc))
        singles = ctx.enter_context(tc.tile_pool(name="singles", bufs=1))
        encpool = ctx.enter_context(tc.tile_pool(name="encp", bufs=3))
        enpool = ctx.enter_context(tc.tile_pool(name="energy", bufs=3))
        smallp = ctx.enter_context(tc.tile_pool(name="small", bufs=2))
        outp = ctx.enter_context(tc.tile_pool(name="outp", bufs=2))
        ps_e = ctx.enter_context(tc.tile_pool(name="ps_e", bufs=2, space="PSUM"))
        ps_sc = ctx.enter_context(tc.tile_pool(name="ps_sc", bufs=2, space="PSUM"))
        ps_h = ctx.enter_context(tc.tile_pool(name="ps_h", bufs=1, space="PSUM"))
        ps_o = ctx.enter_context(tc.tile_pool(name="ps_o", bufs=1, space="PSUM"))

        # first enc DMAs go out before anything else (HWDGE queue stays clear)
        echunks = {}
        for j in range(SLOTS):
            for c0 in range(0, NP[j], CHUNK):
                cw = min(CHUNK, NP[j] - c0)
                et = encpool.tile([H, CHUNK], dt_c, tag="enc")
                nc.sync.dma_start(et[:, :cw], encs[j][:, c0 : c0 + cw])
                echunks[(j, c0)] = et

        # constants on the SWDGE queue, off the enc stream's path
        w1t_sb = singles.tile([H, H], f32)
        nc.gpsimd.dma_start(w1t_sb[:], w1t[:])
        w2t_sb = singles.tile([H, H], dt_c)
        nc.gpsimd.dma_start(w2t_sb[:], w2t[:])
        bvec_sb = singles.tile([H, 1], f32)
        nc.gpsimd.dma_start(bvec_sb[:], bvec[:])
        vvec_sb = singles.tile([H, 1], dt_c)
        nc.gpsimd.dma_start(vvec_sb[:], vvec[:])
        hid_sb = singles.tile([H, SLOTS], f32)
        nc.gpsimd.dma_start(hid_sb[:], hid[:])
        maskt_sb = singles.tile([128, SLOTS, nt_out], f32)
        nc.gpsimd.dma_start(maskt_sb[:], maskt[:])
        ident_sb = singles.tile([128, 128], f32)
        nc.gpsimd.dma_start(ident_sb[:], ident[:])

        ones1 = singles.tile([1, 128], f32)
        nc.vector.memset(ones1[:], 1.0)
        ones_col = singles.tile([128, 1], f32)
        nc.vector.memset(ones_col[:], 1.0)

        # hproj[h,j] = W1 @ hidden_j + b
        ph = ps_h.tile([H, SLOTS], f32, tag="ps_small")
        nc.tensor.matmul(out=ph[:], lhsT=w1t_sb[:], rhs=hid_sb[:], start=True, stop=True)
        hproj = singles.tile([H, SLOTS], f32)
        nc.scalar.activation(hproj[:], ph[:], AF.Identity, bias=bvec_sb[:])

        # ---- hot loop, software-pipelined: group g's v-dots are emitted after
        # group g+1's energy matmuls so the PE never waits on the relu engines.
        groups = []
        for j in range(SLOTS):
            for c0 in range(0, NP[j], CHUNK):
                cw = min(CHUNK, NP[j] - c0)
                for s in range(0, cw, GRP):
                    groups.append((j, c0, s, min(GRP, cw - s)))

        psc_tiles = {}
        for j in range(SLOTS):
            psc_t = ps_sc.tile([128, nt_out], f32, tag="psc")
            psc_tiles[j] = psc_t

        pending = []  # list of (j, en_tile, start_pos, width)

        def flush_pending():
            for pj, pen, ppos, pw in pending:
                for k in range(0, pw, 128):
                    kw = min(128, pw - k)
                    tidx = (ppos + k) // 128
                    nc.tensor.matmul(
                        out=psc_tiles[pj][:kw, tidx : tidx + 1],
                        lhsT=pen[:, k : k + kw],
                        rhs=vvec_sb[:],
                        start=True,
                        stop=True,
                    )
            pending.clear()

        softmax_after = {}  # group index -> slot to run softmax for
        gi_of_slot_last = {}
        for gi, (j, c0, s, sw) in enumerate(groups):
            gi_of_slot_last[j] = gi
        for j, gi in gi_of_slot_last.items():
            softmax_after[gi] = j

        for gi, (j, c0, s, sw) in enumerate(groups):
            et = echunks[(j, c0)]
            pe = ps_e.tile([H, GRP], f32, tag="pe")
            for m in range(0, sw, MMN):
                mw = min(MMN, sw - m)
                nc.tensor.matmul(
                    out=pe[:, m : m + mw],
                    lhsT=w2t_sb[:],
                    rhs=et[:, s + m : s + m + mw],
                    start=True,
                    stop=True,
                )
            en = enpool.tile([H, GRP], dt_c, tag="en")
            if gi % 2 == 0:
                nc.scalar.activation(
                    en[:, :sw], pe[:, :sw], AF.Relu, bias=hproj[:, j : j + 1]
                )
            else:
                nc.vector.tensor_scalar(
                    out=en[:, :sw],
                    in0=pe[:, :sw],
                    scalar1=hproj[:, j : j + 1],
                    scalar2=0.0,
                    op0=mybir.AluOpType.add,
                    op1=mybir.AluOpType.max,
                )
            flush_pending()
            pending.append((j, en, c0 + s, sw))
            if (gi - 1) in softmax_after:
                _softmax_slot(
                    nc, tc, softmax_after[gi - 1], NP, nt_out, psc_tiles, maskt_sb,
                    ident_sb, ones1, ones_col, smallp, outp, ps_h, ps_o, out, AF, f32,
                )
        flush_pending()
        _softmax_slot(
            nc, tc, groups[-1][0], NP, nt_out, psc_tiles, maskt_sb,
            ident_sb, ones1, ones_col, smallp, outp, ps_h, ps_o, out, AF, f32,
        )


def _softmax_slot(nc, tc, j, NP, nt_out, psc_tiles, maskt_sb, ident_sb, ones1,
                  ones_col, smallp, outp, ps_h, ps_o, out, AF, f32):
    """Fused masked softmax + transposed store for one slot."""
    if True:
        if True:
            nv = NP[j] // 128
            psc = psc_tiles[j]
            fmax = smallp.tile([128, 1], f32, tag="fmax")
            nc.vector.reduce_max(fmax[:], psc[:, :nv], axis=mybir.AxisListType.X)
            pmt = ps_h.tile([1, 128], f32, tag="ps_small")
            nc.tensor.transpose(pmt[:], fmax[:], ident_sb[:])
            mrow = smallp.tile([1, 128], f32, tag="mrow")
            nc.vector.tensor_copy(mrow[:], pmt[:])
            negm = smallp.tile([1, 1], f32, tag="negm")
            nc.vector.tensor_reduce(
                negm[:], mrow[:], axis=mybir.AxisListType.X,
                op=mybir.AluOpType.max, negate=True,
            )
            pnb = ps_h.tile([128, 1], f32, tag="ps_small")
            nc.tensor.matmul(out=pnb[:], lhsT=ones1[:], rhs=negm[:], start=True, stop=True)
            negmb = smallp.tile([128, 1], f32, tag="negmb")
            nc.vector.tensor_copy(negmb[:], pnb[:])
            expm = smallp.tile([128, nt_out], f32, tag="expm")
            nc.scalar.activation(expm[:, :nv], psc[:, :nv], AF.Exp, bias=negmb[:])
            nc.vector.tensor_mul(expm[:, :nv], expm[:, :nv], maskt_sb[:, j, :nv])
            psr = ps_h.tile([1, nt_out], f32, tag="ps_small")
            nc.tensor.matmul(
                out=psr[:, :nv], lhsT=ones_col[:], rhs=expm[:, :nv], start=True, stop=True
            )
            srow = smallp.tile([1, nt_out], f32, tag="srow")
            nc.vector.tensor_copy(srow[:, :nv], psr[:, :nv])
            s11 = smallp.tile([1, 1], f32, tag="s11")
            nc.vector.reduce_sum(s11[:], srow[:, :nv], axis=mybir.AxisListType.X)
            nc.vector.reciprocal(s11[:], s11[:])
            prb = ps_h.tile([128, 1], f32, tag="ps_small")
            nc.tensor.matmul(out=prb[:], lhsT=ones1[:], rhs=s11[:], start=True, stop=True)
            recb = smallp.tile([128, 1], f32, tag="recb")
            nc.vector.tensor_copy(recb[:], prb[:])
            attn = smallp.tile([128, nt_out], f32, tag="attn")
            nc.vector.tensor_scalar_mul(attn[:, :nv], expm[:, :nv], recb[:])

            po = ps_o.tile([nt_out, 128], f32, tag="po")
            nc.tensor.transpose(po[:nv, :], attn[:, :nv], ident_sb[:])
            ob = outp.tile([nt_out, 128], f32, tag="ob")
            nc.vector.tensor_copy(ob[:nv, :], po[:nv, :])
            nc.sync.dma_start(out[j, :nv], ob[:nv, :])


def _make_inmaps(hidden, enc, ls, W_attn, b_attn, v, rows, NP, nt_out, dt_name):
    np_c = _np_dt(dt_name)
    f32 = np.float32
    w1t = np.ascontiguousarray(W_attn[:, :H].T).astype(f32)
    w2t = np.ascontiguousarray(W_attn[:, H:].T).astype(np_c)
    bvec = np.ascontiguousarray(b_attn.reshape(H, 1)).astype(f32)
    vvec = np.ascontiguousarray(v.reshape(H, 1)).astype(np_c)
    ident = np.eye(128, dtype=f32)
    tgrid = (np.arange(nt_out)[None, :] * 128 + np.arange(128)[:, None])  # [128, nt]

    in_maps = []
    for i in range(NCORES):
        m = {"w1t": w1t, "w2t": w2t, "bvec": bvec, "vvec": vvec, "ident": ident}
        hid = np.empty((H, SLOTS), f32)
        maskt = np.empty((128, SLOTS, nt_out), f32)
        for j in range(SLOTS):
            b = rows[i][j]
            m[f"enc{j}"] = np.ascontiguousarray(enc[: NP[j], b, :].T).astype(np_c)
            hid[:, j] = hidden[b, :]
            maskt[:, j, :] = (tgrid < int(ls[b])).astype(f32)
        m["hid"] = hid
        m["maskt"] = maskt
        in_maps.append(m)
    return in_maps


def run(inputs, trace=False, **spmd_kwargs):
    hidden = np.asarray(inputs["hidden"], dtype=np.float32)
    enc = np.asarray(inputs["encoder_outputs"], dtype=np.float32)
    ls = np.asarray(inputs["len_seq"]).astype(np.int64)
    W_attn = np.asarray(inputs["W_attn"], dtype=np.float32)
    b_attn = np.asarray(inputs["b_attn"], dtype=np.float32)
    v = np.asarray(inputs["v"], dtype=np.float32)
    t_len = enc.shape[0]
    nt_out = t_len // 128

    rows, NP = _plan(ls, t_len)
    nc = bacc.Bacc("TRN2", target_bir_lowering=False, debug=False)
    _build(nc, NP, nt_out, COMPUTE_DTYPE)
    nc.compile()
    in_maps = _make_inmaps(hidden, enc, ls, W_attn, b_attn, v, rows, NP, nt_out,
                           COMPUTE_DTYPE)
    res = run_bass_kernel_spmd(
        nc, in_maps, core_ids=list(range(NCORES)), trace=trace, **spmd_kwargs
    )

    final = np.zeros((B, 1, t_len), dtype=np.float32)
    for i in range(NCORES):
        o = np.asarray(res.results[i]["out"], dtype=np.float32).reshape(SLOTS, t_len)
        for j in range(SLOTS):
            b = rows[i][j]
            ln = int(ls[b])
            final[b, 0, :ln] = o[j, :ln]
    return final, res


def kernel(**inputs):
    final, _ = run(inputs, trace=False)
    return final


# revision 21
# speedup vs baseline: 1.1299x; 1.1299x over previous
"""Trainium2 Bass kernel for ragged masked attention-score softmax.

Problem (B=32, T=8192, H=128):
    energy[b,t] = relu(W1 @ hidden[b] + W2 @ enc[t,b] + b_attn)   (W_attn = [W1 | W2])
    scores[b,t] = v . energy[b,t]
    out[b,0,:]  = ragged-masked softmax over t < len_seq[b], zeros after.

Strategy (8 NeuronCores, data-parallel over B):
  - Rows are sorted by len desc; slot j on every core takes one row from rank
    group [8j, 8j+8).  The per-slot static position count NP_j = max len in the
    group (rounded to 128), so all cores run one shared graph while skipping
    ~half the positions (ragged lengths are known at trace time).
  - Host passes each core's rows TRANSPOSED ([H, NP_j], H on partitions) so the
    device streams contiguous tiles straight into the TensorEngine with no
    on-device transpose of the big tensor.
  - Per 512-column tile: energy = W2T.T @ encT (PE) -> relu+hproj bias
    (ScalarE, PSUM->SBUF) -> v-dot via PE (energy tile as stationary, v as
    moving operand) accumulating scores[t,1] columns into a PSUM scores tile.
  - Epilogue: masked softmax on the [128, 4, T/128] scores tile (exact max via
    gpsimd partition_all_reduce), PE transpose per slot, DMA out.
"""

from contextlib import ExitStack

import numpy as np

import concourse.bass as bass
import concourse.tile as tile
from concourse import bacc, mybir
from concourse.bass_utils import run_bass_kernel_spmd

B, T, H = 32, 8192, 128
NCORES = 8
SLOTS = B // NCORES  # 4 rows per core
NEG = -1.0e30

# knobs
COMPUTE_DTYPE = "bfloat16"  # "float32" | "bfloat16" for enc/W2/v/energy matmul path
CHUNK = 4096  # positions per DMA (4096 bf16 = 1 MiB)
GRP = 1024  # relu granularity (2 PSUM banks)
MMN = 512  # matmul moving free dim (one PSUM bank in f32)


def _np_dt(name):
    if name == "bfloat16":
        import ml_dtypes

        return np.dtype(ml_dtypes.bfloat16)
    return np.dtype(np.float32)


def _my_dt(name):
    return mybir.dt.bfloat16 if name == "bfloat16" else mybir.dt.float32


def _plan(ls, t_max):
    """Assign rows to (core, slot). Returns rows[core][slot] = b, NP[slot]."""
    order = np.argsort(-np.asarray(ls), kind="stable")
    rows = [[int(order[8 * j + i]) for j in range(SLOTS)] for i in range(NCORES)]
    NP = []
    for j in range(SLOTS):
        mx = int(max(ls[int(order[8 * j + i])] for i in range(NCORES)))
        NP.append(min(((mx + 127) // 128) * 128, t_max))
    return rows, NP


def _build(nc, NP, nt_out, dt_name):
    """Emit the Tile graph. NP: per-slot position counts (mult of 128).
    nt_out: number of 128-wide t-tiles in the output (T/128)."""
    dt_c = _my_dt(dt_name)
    f32 = mybir.dt.float32
    AF = mybir.ActivationFunctionType

    encs = [
        nc.dram_tensor(f"enc{j}", [H, NP[j]], dt_c, kind="ExternalInput").ap()
        for j in range(SLOTS)
    ]
    hid = nc.dram_tensor("hid", [H, SLOTS], f32, kind="ExternalInput").ap()
    w1t = nc.dram_tensor("w1t", [H, H], f32, kind="ExternalInput").ap()
    w2t = nc.dram_tensor("w2t", [H, H], dt_c, kind="ExternalInput").ap()
    bvec = nc.dram_tensor("bvec", [H, 1], f32, kind="ExternalInput").ap()
    vvec = nc.dram_tensor("vvec", [H, 1], dt_c, kind="ExternalInput").ap()
    maskt = nc.dram_tensor("maskt", [128, SLOTS, nt_out], f32, kind="ExternalInput").ap()
    ident = nc.dram_tensor("ident", [128, 128], f32, kind="ExternalInput").ap()
    out = nc.dram_tensor("out", [SLOTS, nt_out, 128], f32, kind="ExternalOutput").ap()

    with ExitStack() as ctx:
        tc = ctx.enter_context(tile.TileContext(nc))
        singles = ctx.enter_context(tc.tile_pool(name="singles", bufs=1))
        encpool = ctx.enter_context(tc.tile_pool(name="encp", bufs=3))
        enpool = ctx.enter_context(tc.tile_pool(name="energy", bufs=3))
        smallp = ctx.enter_context(tc.tile_pool(name="small", bufs=2))
        outp = ctx.enter_context(tc.tile_pool(name="outp", bufs=2))
        ps_e = ctx.enter_context(tc.tile_pool(name="ps_e", bufs=2, space="PSUM"))
        ps_sc = ctx.enter_context(tc.tile_pool(name="ps_sc", bufs=2, space="PSUM"))
        ps_h = ctx.enter_context(tc.tile_pool(name="ps_h", bufs=1, space="PSUM"))
        ps_o = ctx.enter_context(tc.tile_pool(name="ps_o", bufs=1, space="PSUM"))

        # DMA emission order = queue order: w2t + the first enc chunk lead so
        # the first energy matmul can start ~4us in; other consts follow; the
        # remaining enc chunks stream behind.
        w2t_sb = singles.tile([H, H], dt_c)
        nc.sync.dma_start(w2t_sb[:], w2t[:])
        echunks = {}
        first = (0, 0)
        cw0 = min(CHUNK, NP[0])
        et0 = encpool.tile([H, CHUNK], dt_c, tag="enc")
        nc.sync.dma_start(et0[:, :cw0], encs[0][:, :cw0])
        echunks[first] = et0

        vvec_sb = singles.tile([H, 1], dt_c)
        nc.sync.dma_start(vvec_sb[:], vvec[:])
        w1t_sb = singles.tile([H, H], f32)
        nc.sync.dma_start(w1t_sb[:], w1t[:])
        bvec_sb = singles.tile([H, 1], f32)
        nc.sync.dma_start(bvec_sb[:], bvec[:])
        hid_sb = singles.tile([H, SLOTS], f32)
        nc.sync.dma_start(hid_sb[:], hid[:])
        maskt_sb = singles.tile([128, SLOTS, nt_out], f32)
        nc.sync.dma_start(maskt_sb[:], maskt[:])
        ident_sb = singles.tile([128, 128], f32)
        nc.sync.dma_start(ident_sb[:], ident[:])

        for j in range(SLOTS):
            for c0 in range(0, NP[j], CHUNK):
                if (j, c0) in echunks:
                    continue
                cw = min(CHUNK, NP[j] - c0)
                et = encpool.tile([H, CHUNK], dt_c, tag="enc")
                nc.sync.dma_start(et[:, :cw], encs[j][:, c0 : c0 + cw])
                echunks[(j, c0)] = et

        ones1 = singles.tile([1, 128], f32)
        nc.vector.memset(ones1[:], 1.0)
        ones_col = singles.tile([128, 1], f32)
        nc.vector.memset(ones_col[:], 1.0)

        # hproj[h,j] = W1 @ hidden_j + b
        ph = ps_h.tile([H, SLOTS], f32, tag="ps_small")
        nc.tensor.matmul(out=ph[:], lhsT=w1t_sb[:], rhs=hid_sb[:], start=True, stop=True)
        hproj = singles.tile([H, SLOTS], f32)
        nc.scalar.activation(hproj[:], ph[:], AF.Identity, bias=bvec_sb[:])

        # ---- hot loop, software-pipelined: group g's v-dots are emitted after
        # group g+1's energy matmuls so the PE never waits on the relu engines.
        groups = []
        for j in range(SLOTS):
            for c0 in range(0, NP[j], CHUNK):
                cw = min(CHUNK, NP[j] - c0)
                for s in range(0, cw, GRP):
                    groups.append((j, c0, s, min(GRP, cw - s)))

        psc_tiles = {}
        for j in range(SLOTS):
            psc_t = ps_sc.tile([128, nt_out], f32, tag="psc")
            psc_tiles[j] = psc_t

        pending = []  # list of (j, en_tile, start_pos, width)

        def flush_pending():
            for pj, pen, ppos, pw in pending:
                for k in range(0, pw, 128):
                    kw = min(128, pw - k)
                    tidx = (ppos + k) // 128
                    nc.tensor.matmul(
                        out=psc_tiles[pj][:kw, tidx : tidx + 1],
                        lhsT=pen[:, k : k + kw],
                        rhs=vvec_sb[:],
                        start=True,
                        stop=True,
                    )
            pending.clear()

        softmax_after = {}  # group index -> slot to run softmax for
        gi_of_slot_last = {}
        for gi, (j, c0, s, sw) in enumerate(groups):
            gi_of_slot_last[j] = gi
        for j, gi in gi_of_slot_last.items():
            softmax_after[gi] = j

        for gi, (j, c0, s, sw) in enumerate(groups):
            et = echunks[(j, c0)]
            pe = ps_e.tile([H, GRP], f32, tag="pe")
            for m in range(0, sw, MMN):
                mw = min(MMN, sw - m)
                nc.tensor.matmul(
                    out=pe[:, m : m + mw],
                    lhsT=w2t_sb[:],
                    rhs=et[:, s + m : s + m + mw],
                    start=True,
                    stop=True,
                )
            en = enpool.tile([H, GRP], dt_c, tag="en")
            if gi % 2 == 0:
                nc.scalar.activation(
                    en[:, :sw], pe[:, :sw], AF.Relu, bias=hproj[:, j : j + 1]
                )
            else:
                nc.vector.tensor_scalar(
                    out=en[:, :sw],
                    in0=pe[:, :sw],
                    scalar1=hproj[:, j : j + 1],
                    scalar2=0.0,
                    op0=mybir.AluOpType.add,
                    op1=mybir.AluOpType.max,
                )
            flush_pending()
            pending.append((j, en, c0 + s, sw))
            if (gi - 1) in softmax_after:
                _softmax_slot(
                    nc, tc, softmax_after[gi - 1], NP, nt_out, psc_tiles, maskt_sb,
                    ident_sb, ones1, ones_col, smallp, outp, ps_h, ps_o, out, AF, f32,
                )
        flush_pending()
        _softmax_slot(
            nc, tc, groups[-1][0], NP, nt_out, psc_tiles, maskt_sb,
            ident_sb, ones1, ones_col, smallp, outp, ps_h, ps_o, out, AF, f32,
        )


def _softmax_slot(nc, tc, j, NP, nt_out, psc_tiles, maskt_sb, ident_sb, ones1,
                  ones_col, smallp, outp, ps_h, ps_o, out, AF, f32):
    """Fused masked softmax + transposed store for one slot."""
    if True:
        if True:
            nv = NP[j] // 128
            psc = psc_tiles[j]
            fmax = smallp.tile([128, 1], f32, tag="fmax")
            nc.vector.reduce_max(fmax[:], psc[:, :nv], axis=mybir.AxisListType.X)
            pmt = ps_h.tile([1, 128], f32, tag="ps_small")
            nc.tensor.transpose(pmt[:], fmax[:], ident_sb[:])
            mrow = smallp.tile([1, 128], f32, tag="mrow")
            nc.vector.tensor_copy(mrow[:], pmt[:])
            negm = smallp.tile([1, 1], f32, tag="negm")
            nc.vector.tensor_reduce(
                negm[:], mrow[:], axis=mybir.AxisListType.X,
                op=mybir.AluOpType.max, negate=True,
            )
            pnb = ps_h.tile([128, 1], f32, tag="ps_small")
            nc.tensor.matmul(out=pnb[:], lhsT=ones1[:], rhs=negm[:], start=True, stop=True)
            negmb = smallp.tile([128, 1], f32, tag="negmb")
            nc.vector.tensor_copy(negmb[:], pnb[:])
            expm = smallp.tile([128, nt_out], f32, tag="expm")
            nc.scalar.activation(expm[:, :nv], psc[:, :nv], AF.Exp, bias=negmb[:])
            nc.vector.tensor_mul(expm[:, :nv], expm[:, :nv], maskt_sb[:, j, :nv])
            psr = ps_h.tile([1, nt_out], f32, tag="ps_small")
            nc.tensor.matmul(
                out=psr[:, :nv], lhsT=ones_col[:], rhs=expm[:, :nv], start=True, stop=True
            )
            srow = smallp.tile([1, nt_out], f32, tag="srow")
            nc.vector.tensor_copy(srow[:, :nv], psr[:, :nv])
            s11 = smallp.tile([1, 1], f32, tag="s11")
            nc.vector.reduce_sum(s11[:], srow[:, :nv], axis=mybir.AxisListType.X)
            nc.vector.reciprocal(s11[:], s11[:])
            prb = ps_h.tile([128, 1], f32, tag="ps_small")
            nc.tensor.matmul(out=prb[:], lhsT=ones1[:], rhs=s11[:], start=True, stop=True)
            recb = smallp.tile([128, 1], f32, tag="recb")
            nc.vector.tensor_copy(recb[:], prb[:])
            attn = smallp.tile([128, nt_out], f32, tag="attn")
            nc.vector.tensor_scalar_mul(attn[:, :nv], expm[:, :nv], recb[:])

            po = ps_o.tile([nt_out, 128], f32, tag="po")
            nc.tensor.transpose(po[:nv, :], attn[:, :nv], ident_sb[:])
            ob = outp.tile([nt_out, 128], f32, tag="ob")
            nc.vector.tensor_copy(ob[:nv, :], po[:nv, :])
            nc.sync.dma_start(out[j, :nv], ob[:nv, :])


def _make_inmaps(hidden, enc, ls, W_attn, b_attn, v, rows, NP, nt_out, dt_name):
    np_c = _np_dt(dt_name)
    f32 = np.float32
    w1t = np.ascontiguousarray(W_attn[:, :H].T).astype(f32)
    w2t = np.ascontiguousarray(W_attn[:, H:].T).astype(np_c)
    bvec = np.ascontiguousarray(b_attn.reshape(H, 1)).astype(f32)
    vvec = np.ascontiguousarray(v.reshape(H, 1)).astype(np_c)
    ident = np.eye(128, dtype=f32)
    tgrid = (np.arange(nt_out)[None, :] * 128 + np.arange(128)[:, None])  # [128, nt]

    in_maps = []
    for i in range(NCORES):
        m = {"w1t": w1t, "w2t": w2t, "bvec": bvec, "vvec": vvec, "ident": ident}
        hid = np.empty((H, SLOTS), f32)
        maskt = np.empty((128, SLOTS, nt_out), f32)
        for j in range(SLOTS):
            b = rows[i][j]
            m[f"enc{j}"] = np.ascontiguousarray(enc[: NP[j], b, :].T).astype(np_c)
            hid[:, j] = hidden[b, :]
            maskt[:, j, :] = (tgrid < int(ls[b])).astype(f32)
        m["hid"] = hid
        m["maskt"] = maskt
        in_maps.append(m)
    return in_maps


def run(inputs, trace=False, **spmd_kwargs):
    hidden = np.asarray(inputs["hidden"], dtype=np.float32)
    enc = np.asarray(inputs["encoder_outputs"], dtype=np.float32)
    ls = np.asarray(inputs["len_seq"]).astype(np.int64)
    W_attn = np.asarray(inputs["W_attn"], dtype=np.float32)
    b_attn = np.asarray(inputs["b_attn"], dtype=np.float32)
    v = np.asarray(inputs["v"], dtype=np.float32)
    t_len = enc.shape[0]
    nt_out = t_len // 128

    rows, NP = _plan(ls, t_len)
    nc = bacc.Bacc("TRN2", target_bir_lowering=False, debug=False)
    _build(nc, NP, nt_out, COMPUTE_DTYPE)
    nc.compile()
    in_maps = _make_inmaps(hidden, enc, ls, W_attn, b_attn, v, rows, NP, nt_out,
                           COMPUTE_DTYPE)
    res = run_bass_kernel_spmd(
        nc, in_maps, core_ids=list(range(NCORES)), trace=trace, **spmd_kwargs
    )

    final = np.zeros((B, 1, t_len), dtype=np.float32)
    for i in range(NCORES):
        o = np.asarray(res.results[i]["out"], dtype=np.float32).reshape(SLOTS, t_len)
        for j in range(SLOTS):
            b = rows[i][j]
            ln = int(ls[b])
            final[b, 0, :ln] = o[j, :ln]
    return final, res


def kernel(**inputs):
    final, _ = run(inputs, trace=False)
    return final


# revision 24
# speedup vs baseline: 1.1443x; 1.0128x over previous
"""Trainium2 Bass kernel for ragged masked attention-score softmax.

Problem (B=32, T=8192, H=128):
    energy[b,t] = relu(W1 @ hidden[b] + W2 @ enc[t,b] + b_attn)   (W_attn = [W1 | W2])
    scores[b,t] = v . energy[b,t]
    out[b,0,:]  = ragged-masked softmax over t < len_seq[b], zeros after.

Strategy (8 NeuronCores, data-parallel over B):
  - Rows are sorted by len desc; slot j on every core takes one row from rank
    group [8j, 8j+8).  The per-slot static position count NP_j = max len in the
    group (rounded to 128), so all cores run one shared graph while skipping
    ~half the positions (ragged lengths are known at trace time).
  - Host passes each core's rows TRANSPOSED ([H, NP_j], H on partitions) so the
    device streams contiguous tiles straight into the TensorEngine with no
    on-device transpose of the big tensor.
  - Per 512-column tile: energy = W2T.T @ encT (PE) -> relu+hproj bias
    (ScalarE, PSUM->SBUF) -> v-dot via PE (energy tile as stationary, v as
    moving operand) accumulating scores[t,1] columns into a PSUM scores tile.
  - Epilogue: masked softmax on the [128, 4, T/128] scores tile (exact max via
    gpsimd partition_all_reduce), PE transpose per slot, DMA out.
"""

from contextlib import ExitStack

import numpy as np

import concourse.bass as bass
import concourse.tile as tile
from concourse import bacc, mybir
from concourse.bass_utils import run_bass_kernel_spmd

B, T, H = 32, 8192, 128
NCORES = 8
SLOTS = B // NCORES  # 4 rows per core
NEG = -1.0e30

# knobs
COMPUTE_DTYPE = "bfloat16"  # "float32" | "bfloat16" for enc/W2/v/energy matmul path
CHUNK = 4096  # positions per DMA (4096 bf16 = 1 MiB)
GRP = 1024  # relu granularity (2 PSUM banks)
MMN = 512  # matmul moving free dim (one PSUM bank in f32)


def _np_dt(name):
    if name == "bfloat16":
        import ml_dtypes

        return np.dtype(ml_dtypes.bfloat16)
    return np.dtype(np.float32)


def _my_dt(name):
    return mybir.dt.bfloat16 if name == "bfloat16" else mybir.dt.float32


def _plan(ls, t_max):
    """Assign rows to (core, slot). Returns rows[core][slot] = b, NP[slot]."""
    order = np.argsort(-np.asarray(ls), kind="stable")
    rows = [[int(order[8 * j + i]) for j in range(SLOTS)] for i in range(NCORES)]
    NP = []
    for j in range(SLOTS):
        mx = int(max(ls[int(order[8 * j + i])] for i in range(NCORES)))
        NP.append(min(((mx + 127) // 128) * 128, t_max))
    return rows, NP


def _build(nc, NP, nt_out, dt_name):
    """Emit the Tile graph. NP: per-slot position counts (mult of 128).
    nt_out: number of 128-wide t-tiles in the output (T/128)."""
    dt_c = _my_dt(dt_name)
    f32 = mybir.dt.float32
    AF = mybir.ActivationFunctionType

    encs = [
        nc.dram_tensor(f"enc{j}", [H, NP[j]], dt_c, kind="ExternalInput").ap()
        for j in range(SLOTS)
    ]
    hid = nc.dram_tensor("hid", [H, SLOTS], f32, kind="ExternalInput").ap()
    w1t = nc.dram_tensor("w1t", [H, H], f32, kind="ExternalInput").ap()
    w2t = nc.dram_tensor("w2t", [H, H], dt_c, kind="ExternalInput").ap()
    bvec = nc.dram_tensor("bvec", [H, 1], f32, kind="ExternalInput").ap()
    vvec = nc.dram_tensor("vvec", [H, 1], dt_c, kind="ExternalInput").ap()
    maskt = nc.dram_tensor("maskt", [128, SLOTS, nt_out], f32, kind="ExternalInput").ap()
    ident = nc.dram_tensor("ident", [128, 128], f32, kind="ExternalInput").ap()
    out = nc.dram_tensor("out", [SLOTS, nt_out, 128], f32, kind="ExternalOutput").ap()

    with ExitStack() as ctx:
        tc = ctx.enter_context(tile.TileContext(nc))
        singles = ctx.enter_context(tc.tile_pool(name="singles", bufs=1))
        encpool = ctx.enter_context(tc.tile_pool(name="encp", bufs=3))
        enpool = ctx.enter_context(tc.tile_pool(name="energy", bufs=3))
        smallp = ctx.enter_context(tc.tile_pool(name="small", bufs=2))
        outp = ctx.enter_context(tc.tile_pool(name="outp", bufs=2))
        ps_e = ctx.enter_context(tc.tile_pool(name="ps_e", bufs=2, space="PSUM"))
        ps_sc = ctx.enter_context(tc.tile_pool(name="ps_sc", bufs=2, space="PSUM"))
        ps_h = ctx.enter_context(tc.tile_pool(name="ps_h", bufs=1, space="PSUM"))
        ps_o = ctx.enter_context(tc.tile_pool(name="ps_o", bufs=1, space="PSUM"))

        # PE warm-up: dense dummy matmuls during the DMA-wait window release
        # the HAM clock gate (1.2 -> 2.4 GHz) before the real stream begins.
        dum = singles.tile([H, H], dt_c)
        nc.vector.memset(dum[:], 0.0)
        dume = singles.tile([1, 1], f32)
        nc.vector.memset(dume[:], 0.0)
        pdum = ps_h.tile([H, 1], f32, tag="ps_small")
        for _ in range(16):
            nc.tensor.matmul(
                out=pdum[:], lhsT=dum[:], rhs=dum[:, :1], start=True, stop=True
            )
        # preload the exp ACT table set while DMAs stream
        exp_warm = singles.tile([1, 1], f32)
        nc.scalar.activation(exp_warm[:], dume[:], AF.Exp)

        # DMA emission order = queue order: w2t + a small first slice of enc
        # lead so the first energy matmul can start early; critical consts
        # follow; the remaining enc chunks stream behind; softmax-only consts
        # (maskt/ident) ride after slot 0's data.
        w2t_sb = singles.tile([H, H], dt_c)
        nc.sync.dma_start(w2t_sb[:], w2t[:])
        echunks = {}
        cw0 = min(CHUNK, NP[0])
        et0 = encpool.tile([H, CHUNK], dt_c, tag="enc")
        lead = min(GRP, cw0)
        nc.sync.dma_start(et0[:, :lead], encs[0][:, :lead])
        echunks[(0, 0)] = et0

        vvec_sb = singles.tile([H, 1], dt_c)
        nc.sync.dma_start(vvec_sb[:], vvec[:])
        w1t_sb = singles.tile([H, H], f32)
        nc.sync.dma_start(w1t_sb[:], w1t[:])
        bvec_sb = singles.tile([H, 1], f32)
        nc.sync.dma_start(bvec_sb[:], bvec[:])
        hid_sb = singles.tile([H, SLOTS], f32)
        nc.sync.dma_start(hid_sb[:], hid[:])
        if lead < cw0:
            nc.sync.dma_start(et0[:, lead:cw0], encs[0][:, lead:cw0])

        maskt_sb = singles.tile([128, SLOTS, nt_out], f32)
        ident_sb = singles.tile([128, 128], f32)
        for j in range(SLOTS):
            for c0 in range(0, NP[j], CHUNK):
                if (j, c0) in echunks:
                    continue
                cw = min(CHUNK, NP[j] - c0)
                et = encpool.tile([H, CHUNK], dt_c, tag="enc")
                nc.sync.dma_start(et[:, :cw], encs[j][:, c0 : c0 + cw])
                echunks[(j, c0)] = et
            if j == 0:
                nc.sync.dma_start(maskt_sb[:], maskt[:])
                nc.sync.dma_start(ident_sb[:], ident[:])

        ones1 = singles.tile([1, 128], f32)
        nc.vector.memset(ones1[:], 1.0)
        ones_col = singles.tile([128, 1], f32)
        nc.vector.memset(ones_col[:], 1.0)

        # hproj[h,j] = W1 @ hidden_j + b
        ph = ps_h.tile([H, SLOTS], f32, tag="ps_small")
        nc.tensor.matmul(out=ph[:], lhsT=w1t_sb[:], rhs=hid_sb[:], start=True, stop=True)
        hproj = singles.tile([H, SLOTS], f32)
        nc.scalar.activation(hproj[:], ph[:], AF.Identity, bias=bvec_sb[:])

        # ---- hot loop, software-pipelined: group g's v-dots are emitted after
        # group g+1's energy matmuls so the PE never waits on the relu engines.
        groups = []
        for j in range(SLOTS):
            for c0 in range(0, NP[j], CHUNK):
                cw = min(CHUNK, NP[j] - c0)
                for s in range(0, cw, GRP):
                    groups.append((j, c0, s, min(GRP, cw - s)))

        psc_tiles = {}
        for j in range(SLOTS):
            psc_t = ps_sc.tile([128, nt_out], f32, tag="psc")
            psc_tiles[j] = psc_t

        pending = []  # list of (j, en_tile, start_pos, width)

        def flush_pending():
            for pj, pen, ppos, pw in pending:
                for k in range(0, pw, 128):
                    kw = min(128, pw - k)
                    tidx = (ppos + k) // 128
                    nc.tensor.matmul(
                        out=psc_tiles[pj][:kw, tidx : tidx + 1],
                        lhsT=pen[:, k : k + kw],
                        rhs=vvec_sb[:],
                        start=True,
                        stop=True,
                    )
            pending.clear()

        softmax_after = {}  # group index -> slot to run softmax for
        gi_of_slot_last = {}
        for gi, (j, c0, s, sw) in enumerate(groups):
            gi_of_slot_last[j] = gi
        for j, gi in gi_of_slot_last.items():
            softmax_after[gi] = j

        for gi, (j, c0, s, sw) in enumerate(groups):
            et = echunks[(j, c0)]
            pe = ps_e.tile([H, GRP], f32, tag="pe")
            for m in range(0, sw, MMN):
                mw = min(MMN, sw - m)
                nc.tensor.matmul(
                    out=pe[:, m : m + mw],
                    lhsT=w2t_sb[:],
                    rhs=et[:, s + m : s + m + mw],
                    start=True,
                    stop=True,
                )
            en = enpool.tile([H, GRP], dt_c, tag="en")
            if gi % 2 == 0:
                nc.scalar.activation(
                    en[:, :sw], pe[:, :sw], AF.Relu, bias=hproj[:, j : j + 1]
                )
            else:
                nc.vector.tensor_scalar(
                    out=en[:, :sw],
                    in0=pe[:, :sw],
                    scalar1=hproj[:, j : j + 1],
                    scalar2=0.0,
                    op0=mybir.AluOpType.add,
                    op1=mybir.AluOpType.max,
                )
            flush_pending()
            pending.append((j, en, c0 + s, sw))
            if (gi - 1) in softmax_after:
                _softmax_slot(
                    nc, tc, softmax_after[gi - 1], NP, nt_out, psc_tiles, maskt_sb,
                    ident_sb, ones1, ones_col, smallp, outp, ps_h, ps_o, out, AF, f32,
                )
        flush_pending()
        _softmax_slot(
            nc, tc, groups[-1][0], NP, nt_out, psc_tiles, maskt_sb,
            ident_sb, ones1, ones_col, smallp, outp, ps_h, ps_o, out, AF, f32,
        )


def _softmax_slot(nc, tc, j, NP, nt_out, psc_tiles, maskt_sb, ident_sb, ones1,
                  ones_col, smallp, outp, ps_h, ps_o, out, AF, f32):
    """Fused masked softmax + transposed store for one slot."""
    if True:
        if True:
            nv = NP[j] // 128
            psc = psc_tiles[j]
            fmax = smallp.tile([128, 1], f32, tag="fmax")
            nc.vector.reduce_max(fmax[:], psc[:, :nv], axis=mybir.AxisListType.X)
            pmt = ps_h.tile([1, 128], f32, tag="ps_small")
            nc.tensor.transpose(pmt[:], fmax[:], ident_sb[:])
            negm = smallp.tile([1, 1], f32, tag="negm")
            nc.vector.tensor_reduce(
                negm[:], pmt[:], axis=mybir.AxisListType.X,
                op=mybir.AluOpType.max, negate=True,
            )
            pnb = ps_h.tile([128, 1], f32, tag="ps_small")
            nc.tensor.matmul(out=pnb[:], lhsT=ones1[:], rhs=negm[:], start=True, stop=True)
            negmb = smallp.tile([128, 1], f32, tag="negmb")
            nc.vector.tensor_copy(negmb[:], pnb[:])
            expm = smallp.tile([128, nt_out], f32, tag="expm")
            nc.scalar.activation(expm[:, :nv], psc[:, :nv], AF.Exp, bias=negmb[:])
            nc.vector.tensor_mul(expm[:, :nv], expm[:, :nv], maskt_sb[:, j, :nv])
            psr = ps_h.tile([1, nt_out], f32, tag="ps_small")
            nc.tensor.matmul(
                out=psr[:, :nv], lhsT=ones_col[:], rhs=expm[:, :nv], start=True, stop=True
            )
            s11 = smallp.tile([1, 1], f32, tag="s11")
            nc.vector.reduce_sum(s11[:], psr[:, :nv], axis=mybir.AxisListType.X)
            nc.vector.reciprocal(s11[:], s11[:])
            prb = ps_h.tile([128, 1], f32, tag="ps_small")
            nc.tensor.matmul(out=prb[:], lhsT=ones1[:], rhs=s11[:], start=True, stop=True)
            recb = smallp.tile([128, 1], f32, tag="recb")
            nc.vector.tensor_copy(recb[:], prb[:])
            attn = smallp.tile([128, nt_out], f32, tag="attn")
            nc.vector.tensor_scalar_mul(attn[:, :nv], expm[:, :nv], recb[:])

            po = ps_o.tile([nt_out, 128], f32, tag="po")
            nc.tensor.transpose(po[:nv, :], attn[:, :nv], ident_sb[:])
            ob = outp.tile([nt_out, 128], f32, tag="ob")
            nc.vector.tensor_copy(ob[:nv, :], po[:nv, :])
            nc.sync.dma_start(out[j, :nv], ob[:nv, :])


def _make_inmaps(hidden, enc, ls, W_attn, b_attn, v, rows, NP, nt_out, dt_name):
    np_c = _np_dt(dt_name)
    f32 = np.float32
    w1t = np.ascontiguousarray(W_attn[:, :H].T).astype(f32)
    w2t = np.ascontiguousarray(W_attn[:, H:].T).astype(np_c)
    bvec = np.ascontiguousarray(b_attn.reshape(H, 1)).astype(f32)
    vvec = np.ascontiguousarray(v.reshape(H, 1)).astype(np_c)
    ident = np.eye(128, dtype=f32)
    tgrid = (np.arange(nt_out)[None, :] * 128 + np.arange(128)[:, None])  # [128, nt]

    in_maps = []
    for i in range(NCORES):
        m = {"w1t": w1t, "w2t": w2t, "bvec": bvec, "vvec": vvec, "ident": ident}
        hid = np.empty((H, SLOTS), f32)
        maskt = np.empty((128, SLOTS, nt_out), f32)
        for j in range(SLOTS):
            b = rows[i][j]
            m[f"enc{j}"] = np.ascontiguousarray(enc[: NP[j], b, :].T).astype(np_c)
            hid[:, j] = hidden[b, :]
            maskt[:, j, :] = (tgrid < int(ls[b])).astype(f32)
        m["hid"] = hid
        m["maskt"] = maskt
        in_maps.append(m)
    return in_maps


def run(inputs, trace=False, **spmd_kwargs):
    hidden = np.asarray(inputs["hidden"], dtype=np.float32)
    enc = np.asarray(inputs["encoder_outputs"], dtype=np.float32)
    ls = np.asarray(inputs["len_seq"]).astype(np.int64)
    W_attn = np.asarray(inputs["W_attn"], dtype=np.float32)
    b_attn = np.asarray(inputs["b_attn"], dtype=np.float32)
    v = np.asarray(inputs["v"], dtype=np.float32)
    t_len = enc.shape[0]
    nt_out = t_len // 128

    rows, NP = _plan(ls, t_len)
    nc = bacc.Bacc("TRN2", target_bir_lowering=False, debug=False)
    _build(nc, NP, nt_out, COMPUTE_DTYPE)
    nc.compile()
    in_maps = _make_inmaps(hidden, enc, ls, W_attn, b_attn, v, rows, NP, nt_out,
                           COMPUTE_DTYPE)
    res = run_bass_kernel_spmd(
        nc, in_maps, core_ids=list(range(NCORES)), trace=trace, **spmd_kwargs
    )

    final = np.zeros((B, 1, t_len), dtype=np.float32)
    for i in range(NCORES):
        o = np.asarray(res.results[i]["out"], dtype=np.float32).reshape(SLOTS, t_len)
        for j in range(SLOTS):
            b = rows[i][j]
            ln = int(ls[b])
            final[b, 0, :ln] = o[j, :ln]
    return final, res


def kernel(**inputs):
    final, _ = run(inputs, trace=False)
    return final
